# revision 26
# baseline (speedup 1.0000x reference)
"""Trainium2 Bass kernel for nn_DotProductAttention_292057776923.

Per-head windowed attention with valid-length masking:
  out[h] = softmax(Q[h] K[h]^T / sqrt(d) + wmask[w(h)], masked k>=len[h]) @ V[h]
n=256 heads (B2 x W16 x H8), S=512, d=128, f32.

Sharding: pure head-parallel across 8 cores (32 consecutive heads each);
core c needs window masks [4*(c%4), 4*(c%4)+4). No cross-core communication.

Device algorithm (per head, scoresT layout [k, q] so attention never needs
an on-chip transpose of the [512,512] score matrix):
  - PE-transpose Q,K chunks -> QT,KT [d, S] (f32r-rounded on the PSUM->SBUF cast)
  - scoresT[k_tile] = KT_chunk.T @ QT          (f32r matmul, N=512, full rate)
  - E = Exp(scoresT * scale[k] + bias[k])      (ACT; per-partition scale/bias
        implement /sqrt(d) and valid-len replacement with -60)
  - eT = E * exp(wmask)^T                      (GPSIMD; window mask folded in
        exp-domain; exp(wmask)^T built once per window on-device)
  - out_unnorm[q,:128] | sums[q] = eT_chunk.T @ [V | ones | 0pad]  (f32r, N=256)
  - out = out_unnorm * (1/sums)                (DVE reciprocal + ACT scale-copy)

Valid-length truncation: only ceil(len/128) k-tiles contribute (masked tiles
exponentiate to exp(-60) ~ 1e-26 — exactly the reference's zero weights).
Heads are sorted within each 8-head window group by needed tiles and the
SPMD program uses the per-slot max across cores, so one program serves all
8 cores with ~zero waste. len==0 heads (reference: uniform attention) are
overwritten on the host with mean(V) (~0.5 heads expected per run).
"""
import os
import sys

sys.path.insert(0, "/opt/trn_rl_repo")

import numpy as np
from contextlib import ExitStack

import concourse.bass as bass
import concourse.tile as tile
from concourse import bacc, mybir
from concourse.bass_utils import run_bass_kernel_spmd

F32 = mybir.dt.float32
F32R = mybir.dt.float32r
EXP = mybir.ActivationFunctionType.Exp

N, S, D = 256, 512, 128
NT = S // 128            # 4 k/q tiles per head
N_CORES = 8
HPC = N // N_CORES       # 32 heads per core
WPC = 4                  # window groups per core
HPW = HPC // WPC         # 8 heads per window group
MASK_BIAS = -60.0

USE_F32R = os.environ.get("ATTN_F32R", "1") == "1"
TRUNC = os.environ.get("ATTN_TRUNC", "1") == "1"
ET_BF16 = os.environ.get("ATTN_ET_BF16", "0") == "1"
AVN = 256 if USE_F32R else 132
MMDT = F32R if USE_F32R else F32
ETDT = mybir.dt.bfloat16 if ET_BF16 else MMDT
AVDT = ETDT


def _plan(valid_lens):
    """slot_kt[w][i]: k-tiles computed by slot i of window group w (uniform
    across cores); perm[c][s]: head index (within core) assigned to slot s."""
    kt_head = np.maximum(1, np.ceil(valid_lens / 128.0).astype(np.int64))
    if not TRUNC:
        kt_head[:] = NT
    kt_head = kt_head.reshape(N_CORES, WPC, HPW)
    order = np.argsort(-kt_head, axis=2, kind="stable")      # [C, W, 8]
    sorted_kt = np.take_along_axis(kt_head, order, axis=2)   # [C, W, 8]
    slot_kt = sorted_kt.max(axis=0)                          # [W, 8]
    perm = (order + (np.arange(WPC) * HPW)[None, :, None]).reshape(N_CORES, HPC)
    return slot_kt, perm


def _build_program(slot_kt):
    nc = bacc.Bacc("TRN2", target_bir_lowering=False, debug=False,
                   enable_asserts=True, num_devices=N_CORES)
    q_ap = nc.dram_tensor("q", [HPC, S, D], F32, kind="ExternalInput").ap()
    k_ap = nc.dram_tensor("k", [HPC, S, D], F32, kind="ExternalInput").ap()
    v_ap = nc.dram_tensor("v", [HPC, S, D], AVDT, kind="ExternalInput").ap()
    wm_ap = nc.dram_tensor("wm", [WPC, S, S], F32, kind="ExternalInput").ap()
    id_ap = nc.dram_tensor("ident", [128, 128], F32, kind="ExternalInput").ap()
    op_ap = nc.dram_tensor("onespad", [128, 128], AVDT, kind="ExternalInput").ap()
    sc_ap = nc.dram_tensor("scalev", [128, HPC * NT], F32, kind="ExternalInput").ap()
    bi_ap = nc.dram_tensor("biasv", [128, HPC * NT], F32, kind="ExternalInput").ap()
    o_ap = nc.dram_tensor("o", [HPC, S, D], F32, kind="ExternalOutput").ap()

    with tile.TileContext(nc) as tc, ExitStack() as ctx:
        const_p = ctx.enter_context(tc.tile_pool(name="const", bufs=1))
        qkn = ctx.enter_context(tc.tile_pool(name="qkn", bufs=3))
        qkT = ctx.enter_context(tc.tile_pool(name="qkT", bufs=4))
        vxp = ctx.enter_context(tc.tile_pool(name="vxp", bufs=3))
        wmp = ctx.enter_context(tc.tile_pool(name="wmp", bufs=3))
        ewmp = ctx.enter_context(tc.tile_pool(name="ewmp", bufs=8))
        ep = ctx.enter_context(tc.tile_pool(name="ep", bufs=3))
        etp = ctx.enter_context(tc.tile_pool(name="etp", bufs=6))
        obp = ctx.enter_context(tc.tile_pool(name="obp", bufs=3))
        rp = ctx.enter_context(tc.tile_pool(name="rp", bufs=8))
        pt = ctx.enter_context(tc.tile_pool(name="pt", bufs=2, space="PSUM"))
        ps = ctx.enter_context(tc.tile_pool(name="ps", bufs=2, space="PSUM"))
        po = ctx.enter_context(tc.tile_pool(name="po", bufs=2, space="PSUM"))

        ident = const_p.tile([128, 128], F32)
        nc.sync.dma_start(ident[:], id_ap[:])
        onespad = const_p.tile([128, 128], AVDT)
        nc.sync.dma_start(onespad[:], op_ap[:])
        scv = const_p.tile([128, HPC * NT], F32)
        nc.sync.dma_start(scv[:], sc_ap[:])
        biv = const_p.tile([128, HPC * NT], F32)
        nc.sync.dma_start(biv[:], bi_ap[:])

        # prefetch the first two heads' q/k before window-0 mask prep so
        # PE transposes have operands during the mask pipeline warmup
        prefetched = {}
        for s0 in (0, 1):
            kth0 = int(slot_kt[0][s0])
            qn0 = qkn.tile([128, S], F32, name="qn", tag="qn")
            nc.sync.dma_start(qn0[:], q_ap[s0].rearrange("(t p) d -> p t d", p=128))
            kn0 = qkn.tile([128, S], F32, name="kn", tag="kn")
            nc.sync.dma_start(
                kn0[:, 0:kth0*128],
                k_ap[s0, 0:kth0*128, :].rearrange("(t p) d -> p t d", p=128))
            prefetched[s0] = (qn0, kn0)

        for w in range(WPC):
            ktw = int(slot_kt[w].max())
            # ewm[kt] = exp(wmask[w])^T tiles [k=128, q=512], kt < ktw
            ewm = [ewmp.tile([128, S], F32, name="ewm", tag="ewm")
                   for _ in range(ktw)]
            for qt in range(NT):
                wmn = wmp.tile([128, S], F32, name="wmn", tag="wmn")
                nc.sync.dma_start(wmn[:, 0:ktw*128],
                                  wm_ap[w, qt*128:(qt+1)*128, 0:ktw*128])
                e_nat = wmp.tile([128, S], F32, name="e_nat", tag="e_nat")
                nc.scalar.activation(e_nat[:, 0:ktw*128], wmn[:, 0:ktw*128], EXP)
                ptw = pt.tile([128, 512], F32, name="ptw", tag="ptw")
                for kt in range(ktw):
                    nc.tensor.transpose(ptw[:, kt*128:(kt+1)*128],
                                        e_nat[:, kt*128:(kt+1)*128], ident[:])
                for kt in range(ktw):
                    nc.vector.tensor_copy(ewm[kt][:, qt*128:(qt+1)*128],
                                          ptw[:, kt*128:(kt+1)*128])

            for i in range(HPW):
                s = w * HPW + i
                kth = int(slot_kt[w][i])

                if s in prefetched:
                    qn, kn = prefetched[s]
                else:
                    qn = qkn.tile([128, S], F32, name="qn", tag="qn")
                    nc.sync.dma_start(qn[:], q_ap[s].rearrange("(t p) d -> p t d", p=128))
                    kn = qkn.tile([128, S], F32, name="kn", tag="kn")
                    nc.sync.dma_start(
                        kn[:, 0:kth*128],
                        k_ap[s, 0:kth*128, :].rearrange("(t p) d -> p t d", p=128))

                # V tiles: [128, kt, AVN] rows of [V | ones | 0pad]
                vxq = vxp.tile([128, NT * AVN], AVDT, name="vxq", tag="vxq")
                vq = vxq.rearrange("p (t n) -> p t n", n=AVN)
                nc.sync.dma_start(
                    vq[:, 0:kth, 0:128],
                    v_ap[s, 0:kth*128, :].rearrange("(t p) d -> p t d", p=128))
                nc.vector.tensor_copy(
                    vq[:, 0:kth, 128:AVN],
                    onespad[:, 0:AVN-128].unsqueeze(1).broadcast_to(
                        [128, kth, AVN - 128]))

                QT = qkT.tile([128, S], MMDT, name="QT", tag="QT")
                ptq = pt.tile([128, 512], F32, name="ptq", tag="ptw")
                for t in range(NT):
                    nc.tensor.transpose(ptq[:, t*128:(t+1)*128],
                                        qn[:, t*128:(t+1)*128], ident[:])
                nc.vector.tensor_copy(QT[:], ptq[:])

                KT = qkT.tile([128, S], MMDT, name="KT", tag="KT")
                ptk = pt.tile([128, 512], F32, name="ptk", tag="ptw")
                for t in range(kth):
                    nc.tensor.transpose(ptk[:, t*128:(t+1)*128],
                                        kn[:, t*128:(t+1)*128], ident[:])
                nc.vector.tensor_copy(KT[:, 0:kth*128], ptk[:, 0:kth*128])

                # all 4 q-tiles' accumulators in one 2-bank PSUM tile;
                # start=True only on the first matmul touching each bank's
                # zero region, stop=True only on the last one.
                pov = po.tile([128, NT * AVN], F32, name="pov", tag="pov")
                for kt in range(kth):
                    ps_t = ps.tile([128, S], F32, name="ps_t", tag="ps_t")
                    nc.tensor.matmul(ps_t[:], KT[:, kt*128:(kt+1)*128], QT[:],
                                     start=True, stop=True)
                    E_t = ep.tile([128, S], F32, name="E_t", tag="E_t")
                    c = s * NT + kt
                    nc.scalar.activation(E_t[:], ps_t[:], EXP,
                                         bias=biv[:, c:c+1], scale=scv[:, c:c+1])
                    eT = etp.tile([128, S], ETDT, name="eT", tag="eT")
                    nc.gpsimd.tensor_mul(eT[:, 0:320], E_t[:, 0:320],
                                         ewm[kt][:, 0:320])
                    nc.vector.tensor_mul(eT[:, 320:512], E_t[:, 320:512],
                                         ewm[kt][:, 320:512])
                    for qt in range(NT):
                        nc.tensor.matmul(pov[:, qt*AVN:(qt+1)*AVN],
                                         eT[:, qt*128:(qt+1)*128],
                                         vq[:, kt, :],
                                         start=(kt == 0 and qt % 2 == 0),
                                         stop=(kt == kth-1 and qt % 2 == 1))
                povv = pov.rearrange("p (t n) -> p t n", n=AVN)
                r_t = rp.tile([128, NT], F32, name="r_t", tag="r_t")
                nc.vector.reciprocal(r_t[:], povv[:, :, 128])
                ob = obp.tile([128, S], F32, name="ob", tag="ob")
                for qt in range(NT):
                    nc.scalar.mul(ob[:, qt*128:(qt+1)*128],
                                  povv[:, qt, 0:128], r_t[:, qt:qt+1])
                nc.sync.dma_start(
                    o_ap[s].rearrange("(t p) d -> p t d", p=128), ob[:])
    nc.compile()
    return nc


def _make_in_maps(queries, keys, values, valid_lens, window_mask, perm):
    import ml_dtypes
    av_np_dt = ml_dtypes.bfloat16 if ET_BF16 else np.float32
    isd = 1.0 / np.sqrt(np.float32(D))
    ident_np = np.eye(128, dtype=np.float32)
    onespad_np = np.zeros((128, 128), av_np_dt)
    onespad_np[:, 0] = 1.0

    in_maps = []
    for c in range(N_CORES):
        h0 = c * HPC
        hsel = h0 + perm[c]                              # head for each slot
        lens = valid_lens[hsel]
        kg = np.arange(S)
        valid = kg[None, :] < lens[:, None]              # [HPC(slots), S]
        scalev = np.where(valid, isd, 0.0).astype(np.float32)
        biasv = np.where(valid, 0.0, MASK_BIAS).astype(np.float32)
        scalev = scalev.reshape(HPC, NT, 128).transpose(2, 0, 1).reshape(128, HPC * NT)
        biasv = biasv.reshape(HPC, NT, 128).transpose(2, 0, 1).reshape(128, HPC * NT)
        in_maps.append({
            "q": np.ascontiguousarray(queries[hsel]),
            "k": np.ascontiguousarray(keys[hsel]),
            "v": np.ascontiguousarray(values[hsel].astype(av_np_dt)),
            "wm": np.ascontiguousarray(window_mask[4 * (c % 4): 4 * (c % 4) + 4]),
            "ident": ident_np,
            "onespad": onespad_np,
            "scalev": np.ascontiguousarray(scalev),
            "biasv": np.ascontiguousarray(biasv),
        })
    return in_maps


def _install_ntff_hook():
    import types
    if "antenv.axon_hooks" in sys.modules:
        return
    try:
        from trn_agent_boot.trn_boot import _ntff_profile_via_ctypes
        hook = _ntff_profile_via_ctypes('/opt/axon/libaxon_pjrt.so')
    except Exception:
        hook = None
    mod = types.ModuleType("antenv.axon_hooks")
    mod.get_axon_ntff_profile_hook = lambda: hook
    mod.set_axon_ntff_profile_hook = lambda h: None
    sys.modules["antenv.axon_hooks"] = mod
    try:
        import antenv
        antenv.axon_hooks = mod
    except Exception:
        pass


_LAST_RESULTS = {}


def kernel(queries, keys, values, valid_lens, window_mask):
    queries = np.ascontiguousarray(np.asarray(queries, dtype=np.float32))
    keys = np.ascontiguousarray(np.asarray(keys, dtype=np.float32))
    values = np.ascontiguousarray(np.asarray(values, dtype=np.float32))
    valid_lens = np.asarray(valid_lens, dtype=np.int32)
    window_mask = np.ascontiguousarray(np.asarray(window_mask, dtype=np.float32))

    slot_kt, perm = _plan(valid_lens)
    in_maps = _make_in_maps(queries, keys, values, valid_lens, window_mask, perm)
    nc = _build_program(slot_kt)

    trace = os.environ.get("ATTN_TRACE", "0") == "1"
    if trace:
        _install_ntff_hook()
    res = run_bass_kernel_spmd(nc, in_maps, list(range(N_CORES)), trace=trace)
    _LAST_RESULTS["res"] = res

    out = np.empty((N, S, D), np.float32)
    for c in range(N_CORES):
        out[c * HPC + perm[c]] = res.results[c]["o"]

    # len==0 heads: reference softmaxes an all-(-1e6) row -> uniform
    # attention -> mean of V; the device path can't represent that (the
    # window-mask factor survives exp(-60)). ~0.5 heads expected per run.
    for h in np.nonzero(valid_lens == 0)[0]:
        out[int(h)] = values[int(h)].mean(axis=0, keepdims=True)
    return out


# revision 27
# speedup vs baseline: 1.0126x; 1.0126x over previous
"""Trainium2 Bass kernel for nn_DotProductAttention_292057776923.

Per-head windowed attention with valid-length masking:
  out[h] = softmax(Q[h] K[h]^T / sqrt(d) + wmask[w(h)], masked k>=len[h]) @ V[h]
n=256 heads (B2 x W16 x H8), S=512, d=128, f32.

Sharding: pure head-parallel across 8 cores (32 consecutive heads each);
core c needs window masks [4*(c%4), 4*(c%4)+4). No cross-core communication.

Device algorithm (per head, scoresT layout [k, q] so attention never needs
an on-chip transpose of the [512,512] score matrix):
  - PE-transpose Q,K chunks -> QT,KT [d, S] (f32r-rounded on the PSUM->SBUF cast)
  - scoresT[k_tile] = KT_chunk.T @ QT          (f32r matmul, N=512, full rate)
  - E = Exp(scoresT * scale[k] + bias[k])      (ACT; per-partition scale/bias
        implement /sqrt(d) and valid-len replacement with -60)
  - eT = E * exp(wmask)^T                      (GPSIMD; window mask folded in
        exp-domain; exp(wmask)^T built once per window on-device)
  - out_unnorm[q,:128] | sums[q] = eT_chunk.T @ [V | ones | 0pad]  (f32r, N=256)
  - out = out_unnorm * (1/sums)                (DVE reciprocal + ACT scale-copy)

Valid-length truncation: only ceil(len/128) k-tiles contribute (masked tiles
exponentiate to exp(-60) ~ 1e-26 — exactly the reference's zero weights).
Heads are sorted within each 8-head window group by needed tiles and the
SPMD program uses the per-slot max across cores, so one program serves all
8 cores with ~zero waste. len==0 heads (reference: uniform attention) are
overwritten on the host with mean(V) (~0.5 heads expected per run).
"""
import os
import sys

sys.path.insert(0, "/opt/trn_rl_repo")

import numpy as np
from contextlib import ExitStack

import concourse.bass as bass
import concourse.tile as tile
from concourse import bacc, mybir
from concourse.bass_utils import run_bass_kernel_spmd

F32 = mybir.dt.float32
F32R = mybir.dt.float32r
EXP = mybir.ActivationFunctionType.Exp

N, S, D = 256, 512, 128
NT = S // 128            # 4 k/q tiles per head
N_CORES = 8
HPC = N // N_CORES       # 32 heads per core
WPC = 4                  # window groups per core
HPW = HPC // WPC         # 8 heads per window group
MASK_BIAS = -60.0

USE_F32R = os.environ.get("ATTN_F32R", "1") == "1"
TRUNC = os.environ.get("ATTN_TRUNC", "1") == "1"
ET_BF16 = os.environ.get("ATTN_ET_BF16", "0") == "1"
AVN = 256 if USE_F32R else 132
MMDT = F32R if USE_F32R else F32
ETDT = mybir.dt.bfloat16 if ET_BF16 else MMDT
AVDT = ETDT


def _plan(valid_lens):
    """slot_kt[w][i]: k-tiles computed by slot i of window group w (uniform
    across cores); perm[c][s]: head index (within core) assigned to slot s."""
    kt_head = np.maximum(1, np.ceil(valid_lens / 128.0).astype(np.int64))
    if not TRUNC:
        kt_head[:] = NT
    kt_head = kt_head.reshape(N_CORES, WPC, HPW)
    order = np.argsort(-kt_head, axis=2, kind="stable")      # [C, W, 8]
    sorted_kt = np.take_along_axis(kt_head, order, axis=2)   # [C, W, 8]
    slot_kt = sorted_kt.max(axis=0)                          # [W, 8]
    perm = (order + (np.arange(WPC) * HPW)[None, :, None]).reshape(N_CORES, HPC)
    return slot_kt, perm


def _build_program(slot_kt):
    nc = bacc.Bacc("TRN2", target_bir_lowering=False, debug=False,
                   enable_asserts=True, num_devices=N_CORES)
    q_ap = nc.dram_tensor("q", [HPC, S, D], F32, kind="ExternalInput").ap()
    k_ap = nc.dram_tensor("k", [HPC, S, D], F32, kind="ExternalInput").ap()
    v_ap = nc.dram_tensor("v", [HPC, S, D], AVDT, kind="ExternalInput").ap()
    wm_ap = nc.dram_tensor("wm", [WPC, S, S], F32, kind="ExternalInput").ap()
    id_ap = nc.dram_tensor("ident", [128, 128], F32, kind="ExternalInput").ap()
    op_ap = nc.dram_tensor("onespad", [128, 128], AVDT, kind="ExternalInput").ap()
    sc_ap = nc.dram_tensor("scalev", [128, HPC * NT], F32, kind="ExternalInput").ap()
    bi_ap = nc.dram_tensor("biasv", [128, HPC * NT], F32, kind="ExternalInput").ap()
    o_ap = nc.dram_tensor("o", [HPC, S, D], F32, kind="ExternalOutput").ap()

    with tile.TileContext(nc) as tc, ExitStack() as ctx:
        const_p = ctx.enter_context(tc.tile_pool(name="const", bufs=1))
        qkn = ctx.enter_context(tc.tile_pool(name="qkn", bufs=3))
        qkT = ctx.enter_context(tc.tile_pool(name="qkT", bufs=3))
        vxp = ctx.enter_context(tc.tile_pool(name="vxp", bufs=3))
        wmp = ctx.enter_context(tc.tile_pool(name="wmp", bufs=3))
        ewmp = ctx.enter_context(tc.tile_pool(name="ewmp", bufs=8))
        ep = ctx.enter_context(tc.tile_pool(name="ep", bufs=3))
        etp = ctx.enter_context(tc.tile_pool(name="etp", bufs=4))
        obp = ctx.enter_context(tc.tile_pool(name="obp", bufs=3))
        rp = ctx.enter_context(tc.tile_pool(name="rp", bufs=8))
        pt = ctx.enter_context(tc.tile_pool(name="pt", bufs=2, space="PSUM"))
        ps = ctx.enter_context(tc.tile_pool(name="ps", bufs=2, space="PSUM"))
        po = ctx.enter_context(tc.tile_pool(name="po", bufs=2, space="PSUM"))

        ident = const_p.tile([128, 128], F32)
        nc.sync.dma_start(ident[:], id_ap[:])
        onespad = const_p.tile([128, 128], AVDT)
        nc.sync.dma_start(onespad[:], op_ap[:])
        scv = const_p.tile([128, HPC * NT], F32)
        nc.sync.dma_start(scv[:], sc_ap[:])
        biv = const_p.tile([128, HPC * NT], F32)
        nc.sync.dma_start(biv[:], bi_ap[:])

        # prefetch the first two heads' q/k before window-0 mask prep so
        # PE transposes have operands during the mask pipeline warmup
        prefetched = {}
        for s0 in (0, 1):
            kth0 = int(slot_kt[0][s0])
            qn0 = qkn.tile([128, S], F32, name="qn", tag="qn")
            nc.sync.dma_start(qn0[:], q_ap[s0].rearrange("(t p) d -> p t d", p=128))
            kn0 = qkn.tile([128, S], F32, name="kn", tag="kn")
            nc.sync.dma_start(
                kn0[:, 0:kth0*128],
                k_ap[s0, 0:kth0*128, :].rearrange("(t p) d -> p t d", p=128))
            prefetched[s0] = (qn0, kn0)

        for w in range(WPC):
            ktw = int(slot_kt[w].max())
            # ewm[kt] = exp(wmask[w])^T tiles [k=128, q=512], kt < ktw
            ewm = [ewmp.tile([128, S], F32, name="ewm", tag="ewm")
                   for _ in range(ktw)]
            for qt in range(NT):
                wmn = wmp.tile([128, S], F32, name="wmn", tag="wmn")
                nc.sync.dma_start(wmn[:, 0:ktw*128],
                                  wm_ap[w, qt*128:(qt+1)*128, 0:ktw*128])
                e_nat = wmp.tile([128, S], F32, name="e_nat", tag="e_nat")
                nc.scalar.activation(e_nat[:, 0:ktw*128], wmn[:, 0:ktw*128], EXP)
                ptw = pt.tile([128, 512], F32, name="ptw", tag="ptw")
                for kt in range(ktw):
                    nc.tensor.transpose(ptw[:, kt*128:(kt+1)*128],
                                        e_nat[:, kt*128:(kt+1)*128], ident[:])
                for kt in range(ktw):
                    nc.vector.tensor_copy(ewm[kt][:, qt*128:(qt+1)*128],
                                          ptw[:, kt*128:(kt+1)*128])

            for i in range(HPW):
                s = w * HPW + i
                kth = int(slot_kt[w][i])

                if s in prefetched:
                    qn, kn = prefetched[s]
                else:
                    qn = qkn.tile([128, S], F32, name="qn", tag="qn")
                    nc.sync.dma_start(qn[:], q_ap[s].rearrange("(t p) d -> p t d", p=128))
                    kn = qkn.tile([128, S], F32, name="kn", tag="kn")
                    nc.sync.dma_start(
                        kn[:, 0:kth*128],
                        k_ap[s, 0:kth*128, :].rearrange("(t p) d -> p t d", p=128))

                QT = qkT.tile([128, S], MMDT, name="QT", tag="QT")
                ptq = pt.tile([128, 512], F32, name="ptq", tag="ptw")
                for t in range(NT):
                    nc.tensor.transpose(ptq[:, t*128:(t+1)*128],
                                        qn[:, t*128:(t+1)*128], ident[:])
                nc.vector.tensor_copy(QT[:], ptq[:])

                KT = qkT.tile([128, S], MMDT, name="KT", tag="KT")
                ptk = pt.tile([128, 512], F32, name="ptk", tag="ptw")
                for t in range(kth):
                    nc.tensor.transpose(ptk[:, t*128:(t+1)*128],
                                        kn[:, t*128:(t+1)*128], ident[:])
                nc.vector.tensor_copy(KT[:, 0:kth*128], ptk[:, 0:kth*128])

                # V tiles: [128, kt, AVN] rows of [V | ones | 0pad]
                vxq = vxp.tile([128, NT * AVN], AVDT, name="vxq", tag="vxq")
                vq = vxq.rearrange("p (t n) -> p t n", n=AVN)
                nc.sync.dma_start(
                    vq[:, 0:kth, 0:128],
                    v_ap[s, 0:kth*128, :].rearrange("(t p) d -> p t d", p=128))
                nc.vector.tensor_copy(
                    vq[:, 0:kth, 128:AVN],
                    onespad[:, 0:AVN-128].unsqueeze(1).broadcast_to(
                        [128, kth, AVN - 128]))

                # all 4 q-tiles' accumulators in one 2-bank PSUM tile;
                # start=True only on the first matmul touching each bank's
                # zero region, stop=True only on the last one.
                pov = po.tile([128, NT * AVN], F32, name="pov", tag="pov")
                for kt in range(kth):
                    ps_t = ps.tile([128, S], F32, name="ps_t", tag="ps_t")
                    nc.tensor.matmul(ps_t[:], KT[:, kt*128:(kt+1)*128], QT[:],
                                     start=True, stop=True)
                    E_t = ep.tile([128, S], F32, name="E_t", tag="E_t")
                    c = s * NT + kt
                    nc.scalar.activation(E_t[:], ps_t[:], EXP,
                                         bias=biv[:, c:c+1], scale=scv[:, c:c+1])
                    eT = etp.tile([128, S], ETDT, name="eT", tag="eT")
                    nc.gpsimd.tensor_mul(eT[:, 0:320], E_t[:, 0:320],
                                         ewm[kt][:, 0:320])
                    nc.vector.tensor_mul(eT[:, 320:512], E_t[:, 320:512],
                                         ewm[kt][:, 320:512])
                    for qt in range(NT):
                        nc.tensor.matmul(pov[:, qt*AVN:(qt+1)*AVN],
                                         eT[:, qt*128:(qt+1)*128],
                                         vq[:, kt, :],
                                         start=(kt == 0 and qt % 2 == 0),
                                         stop=(kt == kth-1 and qt % 2 == 1))
                povv = pov.rearrange("p (t n) -> p t n", n=AVN)
                r_t = rp.tile([128, NT], F32, name="r_t", tag="r_t")
                nc.vector.reciprocal(r_t[:], povv[:, :, 128])
                ob = obp.tile([128, S], F32, name="ob", tag="ob")
                for qt in range(NT):
                    nc.scalar.mul(ob[:, qt*128:(qt+1)*128],
                                  povv[:, qt, 0:128], r_t[:, qt:qt+1])
                nc.sync.dma_start(
                    o_ap[s].rearrange("(t p) d -> p t d", p=128), ob[:])
    nc.compile()
    return nc


def _make_in_maps(queries, keys, values, valid_lens, window_mask, perm):
    import ml_dtypes
    av_np_dt = ml_dtypes.bfloat16 if ET_BF16 else np.float32
    isd = 1.0 / np.sqrt(np.float32(D))
    ident_np = np.eye(128, dtype=np.float32)
    onespad_np = np.zeros((128, 128), av_np_dt)
    onespad_np[:, 0] = 1.0

    in_maps = []
    for c in range(N_CORES):
        h0 = c * HPC
        hsel = h0 + perm[c]                              # head for each slot
        lens = valid_lens[hsel]
        kg = np.arange(S)
        valid = kg[None, :] < lens[:, None]              # [HPC(slots), S]
        scalev = np.where(valid, isd, 0.0).astype(np.float32)
        biasv = np.where(valid, 0.0, MASK_BIAS).astype(np.float32)
        scalev = scalev.reshape(HPC, NT, 128).transpose(2, 0, 1).reshape(128, HPC * NT)
        biasv = biasv.reshape(HPC, NT, 128).transpose(2, 0, 1).reshape(128, HPC * NT)
        in_maps.append({
            "q": np.ascontiguousarray(queries[hsel]),
            "k": np.ascontiguousarray(keys[hsel]),
            "v": np.ascontiguousarray(values[hsel].astype(av_np_dt)),
            "wm": np.ascontiguousarray(window_mask[4 * (c % 4): 4 * (c % 4) + 4]),
            "ident": ident_np,
            "onespad": onespad_np,
            "scalev": np.ascontiguousarray(scalev),
            "biasv": np.ascontiguousarray(biasv),
        })
    return in_maps


def _install_ntff_hook():
    import types
    if "antenv.axon_hooks" in sys.modules:
        return
    try:
        from trn_agent_boot.trn_boot import _ntff_profile_via_ctypes
        hook = _ntff_profile_via_ctypes('/opt/axon/libaxon_pjrt.so')
    except Exception:
        hook = None
    mod = types.ModuleType("antenv.axon_hooks")
    mod.get_axon_ntff_profile_hook = lambda: hook
    mod.set_axon_ntff_profile_hook = lambda h: None
    sys.modules["antenv.axon_hooks"] = mod
    try:
        import antenv
        antenv.axon_hooks = mod
    except Exception:
        pass


_LAST_RESULTS = {}


def kernel(queries, keys, values, valid_lens, window_mask):
    queries = np.ascontiguousarray(np.asarray(queries, dtype=np.float32))
    keys = np.ascontiguousarray(np.asarray(keys, dtype=np.float32))
    values = np.ascontiguousarray(np.asarray(values, dtype=np.float32))
    valid_lens = np.asarray(valid_lens, dtype=np.int32)
    window_mask = np.ascontiguousarray(np.asarray(window_mask, dtype=np.float32))

    slot_kt, perm = _plan(valid_lens)
    in_maps = _make_in_maps(queries, keys, values, valid_lens, window_mask, perm)
    nc = _build_program(slot_kt)

    trace = os.environ.get("ATTN_TRACE", "0") == "1"
    if trace:
        _install_ntff_hook()
    res = run_bass_kernel_spmd(nc, in_maps, list(range(N_CORES)), trace=trace)
    _LAST_RESULTS["res"] = res

    out = np.empty((N, S, D), np.float32)
    for c in range(N_CORES):
        out[c * HPC + perm[c]] = res.results[c]["o"]

    # len==0 heads: reference softmaxes an all-(-1e6) row -> uniform
    # attention -> mean of V; the device path can't represent that (the
    # window-mask factor survives exp(-60)). ~0.5 heads expected per run.
    for h in np.nonzero(valid_lens == 0)[0]:
        out[int(h)] = values[int(h)].mean(axis=0, keepdims=True)
    return out
